# revision 43
# baseline (speedup 1.0000x reference)
"""Multi-head attention (B=4, S=2048, D=1024, H=16) on 8 TRN2 NeuronCores.

Sharding: core c handles batch b = c//2 and head-group hg = c%2 (8 heads).
Tensor-parallel within the core pair of a batch: w_q/w_k/w_v column-split,
w_o row-split; host sums the two partial out-projections per batch.

Design notes:
- The ScalarE exp pipe is the roofline: 256 x [128,1024] Exp tiles at
  ~996ns sustained = 255us. Everything else is engineered to hide under it.
- Bias elimination: softmax is invariant to k-constant shifts, so the K bias
  drops out; the V bias is folded into b_o on the host (b_o + w_o @ b_v);
  the Q bias is added on the Vector engine (per-partition tensor_scalar).
- AV uses column-tiled concurrent matmuls (head A -> PE cols 0-63, head B ->
  64-127, each K=128/M=64 with its own moving stream): one 512-cycle pass
  per k-tile for the pair.
- Softmax denominators: DVE sums et tiles into 4 accumulators (3 adds each,
  no copies); M=128 all-ones matmuls both reduce over partitions AND
  broadcast the sums to every output partition; recip at partition base 0
  (the fast-recip DVE op is broken at other bases) + a 0-stride DMA
  broadcast for the high half; one PSUM*SBUF multiply normalizes straight
  out of the accumulator. The whole chain is deferred into the next pair's
  kt loop so the in-order PE queue never waits on it.
- Loop order ch-outer/pair-inner. Weights are host-preshuffled to [128, .]
  so each loads in one full-rate DMA descriptor; x tensors load in
  [128,1024] halves (2KB lines) in consumption order. V-projection is woven
  into pair 0 with fully deferred AV; K o-tiles 1-3, later Q chunks, and
  the previous chunk's out-projection are woven under the exp stream.
"""

import numpy as np
import ml_dtypes
from contextlib import ExitStack

import concourse.bass as bass
import concourse.tile as tile
from concourse import bacc, mybir
from concourse.bass_utils import run_bass_kernel_spmd

BF16 = ml_dtypes.bfloat16
F32 = np.float32

D = 1024
N_HEAD = 16
DH = 64
HPC = 8          # heads per core
HW = HPC * DH    # head-group width = 512
P = 128

TRACE = False    # set by test.py for profiling runs

_PROG = {}


def _bcast_dma(nc, dst, src_row, engine=None):
    """Broadcast a [1, W] SBUF row to [N, W] via a 0-stride free-dim DMA."""
    n = dst.shape[0]
    src_b = bass.AP(tensor=src_row.tensor, offset=src_row.offset,
                    ap=[list(src_row.ap[0]), [0, n], list(src_row.ap[1])])
    (engine or nc.sync).dma_start(dst, src_b)


def _build_program(S):
    dt = mybir.dt
    bf = dt.bfloat16
    f32 = dt.float32

    CH = min(512, S)         # q-chunk width
    NCH = S // CH            # q-chunks (4)
    NT = S // P              # k-tiles (16)
    NI = D // P              # contraction tiles over model dim (8)
    NP = HPC // 2            # head pairs (4)
    NO = D // P              # out-proj o-tiles (8)
    NOQ = HW // P            # q/k-proj o-tiles (4)

    nc = bacc.Bacc("TRN2", target_bir_lowering=False, debug=False)

    xq = nc.dram_tensor("xq", [D, S], bf, kind="ExternalInput")
    xk = nc.dram_tensor("xk", [D, S], bf, kind="ExternalInput")
    xv = nc.dram_tensor("xv", [D, S], bf, kind="ExternalInput")
    wq = nc.dram_tensor("wq", [P, NI * HW], bf, kind="ExternalInput")
    wk = nc.dram_tensor("wk", [P, NI * HW], bf, kind="ExternalInput")
    wv = nc.dram_tensor("wv", [P, NI * HW], bf, kind="ExternalInput")
    bq = nc.dram_tensor("bq", [P, NOQ], f32, kind="ExternalInput")
    wo = nc.dram_tensor("wo", [P, NOQ * D], bf, kind="ExternalInput")
    yT = nc.dram_tensor("yT", [D, S], f32, kind="ExternalOutput")

    AF = mybir.ActivationFunctionType

    with tile.TileContext(nc) as tc:
        with ExitStack() as ctx:
            consts = ctx.enter_context(tc.tile_pool(name="consts", bufs=1))
            wpool = ctx.enter_context(tc.tile_pool(name="wpool", bufs=1))
            xpool = ctx.enter_context(tc.tile_pool(name="xpool", bufs=8))
            slabs = ctx.enter_context(tc.tile_pool(name="slabs", bufs=1))
            epool = ctx.enter_context(tc.tile_pool(name="epool", bufs=7))
            espool = ctx.enter_context(tc.tile_pool(name="espool", bufs=4))
            npool = ctx.enter_context(tc.tile_pool(name="npool", bufs=1))
            spool = ctx.enter_context(tc.tile_pool(name="spool", bufs=2))
            pssc = ctx.enter_context(
                tc.tile_pool(name="pssc", bufs=2, space="PSUM"))
            psmix = ctx.enter_context(
                tc.tile_pool(name="psmix", bufs=2, space="PSUM"))

            # ---- constants ----
            ones1 = consts.tile([P, 1], bf)
            nc.vector.memset(ones1[:], 1.0)
            ones_rowf = consts.tile([1, P], f32)
            nc.vector.memset(ones_rowf[:], 1.0)
            bq_sb = consts.tile([P, NOQ], f32)
            nc.sync.dma_start(bq_sb[:], bq.ap())

            # ---- weights ----
            wq_sb = wpool.tile([P, NI, HW], bf)
            wk_sb = wpool.tile([P, NI, HW], bf)
            wv_sb = wpool.tile([P, NI, HW], bf)
            wo_sb = wpool.tile([P, NOQ, D], bf)

            # ---- persistent activation slabs ----
            k_slab = slabs.tile([P, NP, S], bf)
            v_sb = slabs.tile([P, NT, HW], bf)
            # double-buffered over chunks (ring on ch % 2)
            q_slab = slabs.tile([P, NOQ, 2, CH], bf)
            attn_sb = [slabs.tile([P, 2, CH], bf, name=f"attn{pp}")
                       for pp in range(NP)]

            # ---------- DMA loads ----------
            xk_t = [xpool.tile([P, S], bf, tag="xk", name=f"xkt{i}")
                    for i in range(NI)]
            xq_t = [xpool.tile([P, S], bf, tag="xq", name=f"xqt{i}")
                    for i in range(NI)]
            xv_t = [xpool.tile([P, S], bf, tag="xv", name=f"xvt{i}")
                    for i in range(NI)]

            def ld_xh(xt, xd, i, half):
                # [128, 1024] halves: 2KB partition lines = full DMA rate
                hsl = slice(half * 2 * CH, (half + 1) * 2 * CH)
                nc.sync.dma_start(xt[i][:, hsl],
                                  xd.ap()[i * P:(i + 1) * P, hsl])

            # need-ordered for the exp stream; weights are one DMA each
            nc.sync.dma_start(wk_sb[:], wk.ap())
            for i in range(NI):
                ld_xh(xk_t, xk, i, 0)
            nc.sync.dma_start(wq_sb[:], wq.ap())
            for i in range(NI):
                ld_xh(xq_t, xq, i, 0)
            for i in range(NI):
                ld_xh(xk_t, xk, i, 1)
            nc.sync.dma_start(wv_sb[:], wv.ap())
            for i in range(NI):
                ld_xh(xv_t, xv, i, 0)
            for i in range(NI):
                ld_xh(xv_t, xv, i, 1)
            for i in range(NI):
                ld_xh(xq_t, xq, i, 1)
            nc.sync.dma_start(wo_sb[:], wo.ap())

            # ---------- projection helpers (one [128, CH] tile each) --------
            def kproj(o, chk):
                csl = slice(chk * CH, (chk + 1) * CH)
                ps = psmix.tile([P, CH], f32, tag="mix", name="kp")
                for i in range(NI):
                    nc.tensor.matmul(ps[:], lhsT=wk_sb[:, i, o * P:(o + 1) * P],
                                     rhs=xk_t[i][:, csl],
                                     start=(i == 0), stop=(i == NI - 1))
                nc.vector.tensor_copy(k_slab[:, o, csl], ps[:])

            def qproj(o, chk):
                csl = slice(chk * CH, (chk + 1) * CH)
                ps = psmix.tile([P, CH], f32, tag="mix", name="qp")
                for i in range(NI):
                    nc.tensor.matmul(ps[:], lhsT=wq_sb[:, i, o * P:(o + 1) * P],
                                     rhs=xq_t[i][:, csl],
                                     start=(i == 0), stop=(i == NI - 1))
                nc.vector.tensor_scalar(q_slab[:, o, chk % 2, :], ps[:],
                                        bq_sb[:, o:o + 1], None,
                                        mybir.AluOpType.add)

            def vproj(t):
                tsl = slice(t * P, (t + 1) * P)
                ps = psmix.tile([P, HW], f32, tag="mix", name="vp")
                for i in range(NI):
                    nc.tensor.matmul(ps[:], lhsT=xv_t[i][:, tsl],
                                     rhs=wv_sb[:, i, :],
                                     start=(i == 0), stop=(i == NI - 1))
                nc.vector.tensor_copy(v_sb[:, t, :], ps[:])

            def oproj(o, chk):
                csl = slice(chk * CH, (chk + 1) * CH)
                ps = psmix.tile([P, CH], f32, tag="mix", name="op")
                for c in range(NOQ):
                    nc.tensor.matmul(ps[:], lhsT=wo_sb[:, c, o * P:(o + 1) * P],
                                     rhs=attn_sb[c][:, chk % 2, :],
                                     start=(c == 0), stop=(c == NOQ - 1))
                st = spool.tile([P, CH], f32, tag="stage")
                if chk == NCH - 1 and o % 2 == 1:
                    nc.scalar.copy(st[:], ps[:])
                else:
                    nc.vector.tensor_copy(st[:], ps[:])
                nc.sync.dma_start(yT.ap()[o * P:(o + 1) * P, csl], st[:])

            # ---------- prologue (critical path only) ----------
            kproj(0, 0)
            qproj(0, 0)

            # weave map kt -> [closures] emitted into the attention kt loops
            def weave_for(ch, p):
                wv_at = {}

                def put(kt, fn):
                    wv_at.setdefault(kt, []).append(fn)

                if ch == 0:
                    if p == 0:
                        # all of V (AV consumes v tile kt in-order) plus the
                        # remaining K chunks just before scores need them
                        for t in range(NT):
                            put(min(t + 1, NT - 1), lambda tt=t: vproj(tt))
                        for chk in range(1, NCH):
                            put(4 * chk - 2, lambda c=chk: kproj(0, c))
                        for chk in range(NCH):
                            put(2 * chk + 1, lambda c=chk: kproj(1, c))
                        put(10, lambda: qproj(1, 0))
                    elif p < NP - 1:
                        for chk in range(NCH):
                            put(3 * chk + 2, lambda o=p + 1, c=chk:
                                kproj(o, c))
                        put(10, lambda o=p + 1: qproj(o, 0))
                else:
                    # out-projection of the previous chunk
                    put(6, lambda o=2 * p, c=ch - 1: oproj(o, c))
                    put(10, lambda o=2 * p + 1, c=ch - 1: oproj(o, c))
                if ch < NCH - 1:
                    # Q o-tile p of the next chunk, one chunk ahead
                    put(13, lambda o=p, c=ch + 1: qproj(o, c))
                return wv_at

            # ---------- attention ----------
            # Deferred normalize: denominator matmuls + recip/broadcast of
            # pair i are emitted a few kt into pair i+1 so the in-order PE
            # queue never stalls on the DVE etsum adds.
            pending_norm = [None, None]

            def flush_norm(stage=None):
                for s in ((0, 1) if stage is None else (stage,)):
                    if pending_norm[s] is not None:
                        pending_norm[s]()
                        pending_norm[s] = None

            for ch in range(NCH):
                for p in range(NP):
                    wv_at = weave_for(ch, p)

                    acc = pssc.tile([P, CH], f32, tag="acc", bufs=2,
                                    name="acc")
                    esacc = [espool.tile([P, 2 * CH], bf, tag="es",
                                         name=f"es{j}") for j in range(4)]
                    # etsum: esacc[j] = et[4j]+et[4j+1]; += et[4j+2];
                    # += et[4j+3] -- 12 DVE adds, no copies, no final tree
                    esops = {}
                    for j in range(4):
                        esops.setdefault(4 * j + 2, []).append(
                            (j, 4 * j, 4 * j + 1))
                        esops.setdefault(4 * j + 3, []).append((j, 4 * j + 2))
                        esops.setdefault(4 * j + 4, []).append((j, 4 * j + 3))
                    et_by = {}

                    def emit_esops(kt):
                        for op in esops.get(kt, ()):
                            if len(op) == 3:
                                j, a, b = op
                                nc.vector.tensor_add(esacc[j][:], et_by[a][:],
                                                     et_by[b][:])
                            else:
                                j, a = op
                                nc.vector.tensor_add(esacc[j][:], esacc[j][:],
                                                     et_by[a][:])
                    pend = []

                    def issue_av(et, kt):
                        vb = v_sb[:, kt, p * P:p * P + 64]
                        nc.tensor.matmul(
                            acc[0:64, :], lhsT=vb, rhs=et[:, 0:CH],
                            start=(kt == 0), stop=(kt == NT - 1),
                            tile_position=(0, 0))
                        vb2 = v_sb[:, kt, p * P + 64:(p + 1) * P]
                        nc.tensor.matmul(
                            acc[64:128, :], lhsT=vb2, rhs=et[:, CH:2 * CH],
                            start=(kt == 0), stop=(kt == NT - 1),
                            tile_position=(0, 64))

                    for kt in range(NT):
                        ksl = slice(kt * P, (kt + 1) * P)
                        ps = pssc.tile([P, 2 * CH], f32, tag="sc", name="sc")
                        nc.tensor.matmul(
                            ps[:, 0:CH],
                            lhsT=k_slab[0:64, p, ksl],
                            rhs=q_slab[0:64, p, ch % 2, :],
                            start=True, stop=True, tile_position=(0, 0))
                        nc.tensor.matmul(
                            ps[:, CH:2 * CH],
                            lhsT=k_slab[64:128, p, ksl],
                            rhs=q_slab[64:128, p, ch % 2, :],
                            start=True, stop=True, tile_position=(64, 0))
                        et = epool.tile([P, 2 * CH], bf, tag="exp", name="et")
                        nc.scalar.activation(et[:], ps[:], AF.Exp, scale=0.125)
                        et_by[kt] = et
                        pend.append((et, kt))
                        if len(pend) == 3:
                            e0, k0 = pend.pop(0)
                            issue_av(e0, k0)
                        if kt == 1:
                            flush_norm(0)
                        elif kt == 3:
                            flush_norm(1)
                        emit_esops(kt)
                        for item in wv_at.get(kt, ()):
                            item()
                    for e0, k0 in pend:
                        issue_av(e0, k0)
                    emit_esops(NT)

                    def make_norm(acc=acc, esacc=esacc, p=p, ch=ch):
                        # reciprocal_approx_fast only works at partition base
                        # 0: recip the M=1 denominator rows at base 0, then
                        # K=1 col-tiled ones matmuls replicate them to
                        # partitions 0-63 / 64-127 (the PE broadcasts for
                        # free), DVE copies assemble rcpb, one PSUM*SBUF
                        # multiply normalizes. Fully on-chip, no DMA.
                        state = {}

                        def norm_a():
                            psda = psmix.tile([1, CH], f32, tag="mix",
                                              name="psda")
                            psdb = psmix.tile([1, CH], f32, tag="mix",
                                              name="psdb")
                            for j in range(4):
                                nc.tensor.matmul(psda[0:1, :], lhsT=ones1[:],
                                                 rhs=esacc[j][:, 0:CH],
                                                 start=(j == 0), stop=(j == 3),
                                                 tile_position=(0, 0))
                            for j in range(4):
                                nc.tensor.matmul(psdb[0:1, :], lhsT=ones1[:],
                                                 rhs=esacc[j][:, CH:2 * CH],
                                                 start=(j == 0), stop=(j == 3),
                                                 tile_position=(0, 0))
                            rcpta = npool.tile([1, CH], f32, tag="rcpt",
                                               name="rcpta")
                            nc.vector.reciprocal_approx_fast(rcpta[:],
                                                             psda[0:1, :])
                            rcptb = npool.tile([1, CH], f32, tag="rcpt",
                                               name="rcptb")
                            nc.vector.reciprocal_approx_fast(rcptb[:],
                                                             psdb[0:1, :])
                            state["rcpta"], state["rcptb"] = rcpta, rcptb

                        def norm_b():
                            bca = psmix.tile([P, CH], f32, tag="mix",
                                             name="bca")
                            nc.tensor.matmul(bca[0:64, :],
                                             lhsT=ones_rowf[0:1, 0:64],
                                             rhs=state["rcpta"][0:1, :],
                                             start=True, stop=True,
                                             tile_position=(0, 0))
                            bcb = psmix.tile([P, CH], f32, tag="mix",
                                             name="bcb")
                            nc.tensor.matmul(bcb[64:128, :],
                                             lhsT=ones_rowf[0:1, 0:64],
                                             rhs=state["rcptb"][0:1, :],
                                             start=True, stop=True,
                                             tile_position=(0, 64))
                            rcpb = npool.tile([P, CH], f32, tag="rcpb")
                            nc.vector.tensor_copy(rcpb[0:64, :], bca[0:64, :])
                            nc.vector.tensor_copy(rcpb[64:128, :],
                                                  bcb[64:128, :])
                            nc.vector.tensor_mul(attn_sb[p][:, ch % 2, :],
                                                 acc[:], rcpb[:])
                        return norm_a, norm_b

                    pending_norm[0], pending_norm[1] = make_norm()

            # ---------- tail: out-projection of the last chunk ----------
            flush_norm()
            for o in range(NO):
                oproj(o, NCH - 1)

    nc.compile()
    return nc


def _get_program(S):
    if S not in _PROG:
        _PROG[S] = _build_program(S)
    return _PROG[S]


def enable_trace():
    """Register the NTFF profiling hook (axon images lack antenv.axon_hooks)
    and neuter the cloud artifact upload; then TRACE=True runs return
    exec_time_ns."""
    global TRACE
    import sys
    import types
    import antenv
    if "antenv.axon_hooks" not in sys.modules:
        _m = types.ModuleType("antenv.axon_hooks")
        _m._hook = None
        _m.set_axon_ntff_profile_hook = lambda h: setattr(_m, "_hook", h)
        _m.get_axon_ntff_profile_hook = lambda: _m._hook
        sys.modules["antenv.axon_hooks"] = _m
        antenv.axon_hooks = _m
        from trn_agent_boot.trn_boot import _ntff_profile_via_ctypes
        _m._hook = _ntff_profile_via_ctypes("/opt/axon/libaxon_pjrt.so")
    import concourse.bass_utils as bu
    bu.upload_artifacts = lambda tmpdir: tmpdir
    TRACE = True


def _shuf(wT):
    # [n*128, W] -> [128, n*W]: i-tile-major per partition, one dense DMA
    n = wT.shape[0] // P
    return np.ascontiguousarray(
        wT.reshape(n, P, wT.shape[1]).transpose(1, 0, 2).reshape(P, -1))


def _prep_core_inputs(q, k, v, w_q, b_q, w_k, w_v, b, hg, S):
    hsl = slice(hg * HW, (hg + 1) * HW)
    return {
        "xq": np.ascontiguousarray(q[b].T).astype(BF16),
        "xk": np.ascontiguousarray(k[b].T).astype(BF16),
        "xv": np.ascontiguousarray(v[b].T).astype(BF16),
        "wq": _shuf(w_q[hsl].T).astype(BF16),
        "wk": _shuf(w_k[hsl].T).astype(BF16),
        "wv": _shuf(w_v[hsl].T).astype(BF16),
        "bq": np.ascontiguousarray(b_q[hsl].reshape(HW // P, P).T).astype(F32),
    }


def kernel(q, k, v, w_q, b_q, w_k, b_k, w_v, b_v, w_o, b_o):
    q, k, v = (np.asarray(a, F32) for a in (q, k, v))
    w_q, b_q, w_k, b_k = (np.asarray(a, F32) for a in (w_q, b_q, w_k, b_k))
    w_v, b_v, w_o, b_o = (np.asarray(a, F32) for a in (w_v, b_v, w_o, b_o))
    B, S, _ = q.shape

    nc = _get_program(S)

    # softmax(s + const_over_k) == softmax(s): b_k drops out entirely.
    # b_v contributes attn @ 1 * b_v = b_v (rows sum to 1), folded into b_o.
    b_o_eff = b_o + w_o @ b_v

    n_cores = 2 * B
    in_maps = []
    for c in range(n_cores):
        b, hg = c // 2, c % 2
        m = _prep_core_inputs(q, k, v, w_q, b_q, w_k, w_v, b, hg, S)
        hsl = slice(hg * HW, (hg + 1) * HW)
        m["wo"] = _shuf(w_o[:, hsl].T).astype(BF16)
        in_maps.append(m)

    res = run_bass_kernel_spmd(nc, in_maps, list(range(n_cores)), trace=TRACE)

    out = np.empty((B, S, D), F32)
    for b in range(B):
        yt = res.results[2 * b]["yT"] + res.results[2 * b + 1]["yT"]
        out[b] = yt.T + b_o_eff
    if TRACE:
        kernel.last_exec_time_ns = res.exec_time_ns
    return out
